# revision 13
# baseline (speedup 1.0000x reference)
"""Trainium2 Bass kernel for nn_Decoder (dense transformer, B=8,T=512,D=1024,H=16,L=4,C=10).

Sharding: data-parallel over batch — core b computes batch element b end-to-end.

v2: fp8e4m3 DoubleRow matmuls for the q1/k1/v1/q2 projections (4x PE rate,
weights pre-scaled x512 on host, activations x16 via the transpose copies),
bf16 FFN, per-head-group interleave of linears with attention so the Exp
work on the Activation engine overlaps PE, causal mask via post-exp
affine_select on the (otherwise idle) GPSIMD engine, layernorms fused
per-t-tile into the attention/FFN tails.
"""
import sys
sys.path.insert(0, '/opt/trn_rl_repo')

import numpy as np

T, D, H, DH, L, C = 512, 1024, 16, 64, 4, 10
NT, ND = T // 128, D // 128          # 4 t-tiles, 8 d-tiles
EPS = 1e-5
ISCALE = 1.0 / 32.0                  # 1/sqrt(D)
MAGIC = 0x5F3759DF                   # rsqrt Newton seed
XS = 1.0                             # fp8 activation scale (unused: scale-free cast)
WS = 512.0                           # fp8 weight scale
DS = 1.0 / (XS * WS)                 # descale for fp8 linear outputs

_cache = {}


def _build(has_gb, reps=1):
    import contextlib
    import concourse.bass as bass
    from concourse import bacc, tile, mybir
    from concourse.masks import make_identity

    f32 = mybir.dt.float32
    f32r = mybir.dt.float32r
    bf16 = mybir.dt.bfloat16
    f16 = mybir.dt.float16
    f8 = mybir.dt.float8e4
    i32 = mybir.dt.int32
    AF = mybir.ActivationFunctionType
    AL = mybir.AluOpType
    DR = mybir.MatmulPerfMode.DoubleRow

    nc = bacc.Bacc("TRN2", target_bir_lowering=False, debug=False, num_devices=8)

    x_d = nc.dram_tensor("x", [T, D], f32, kind="ExternalInput").ap()
    ktc_d = nc.dram_tensor("ktc", [D, T], bf16, kind="ExternalInput").ap()
    vac_d = nc.dram_tensor("vac", [T, H * 65], f16, kind="ExternalInput").ap()
    # fp8 weights pre-arranged on host as [L, 4 chunks, 128 partitions, ND*256]
    w8_d = {
        m: nc.dram_tensor(f"w{m}", [L, 4, 128, ND * 256], f8,
                          kind="ExternalInput").ap()
        for m in ("q1", "k1", "v1", "q2")
    }
    wf_d = nc.dram_tensor("wfc", [L, 4, 128, ND * 256], bf16,
                          kind="ExternalInput").ap()
    bfc_d = nc.dram_tensor("bfc", [L, D], bf16, kind="ExternalInput").ap()
    bcut_d = nc.dram_tensor("bcut", [C - 1], f32, kind="ExternalInput").ap()
    if has_gb:
        gb_d = nc.dram_tensor("gb", [3 * L - 1, 2, D], f32, kind="ExternalInput").ap()
    probs_d = nc.dram_tensor("probs", [T, D, C], f32, kind="ExternalOutput").ap()

    def bcast(src_ap, parts=128):
        return bass.AP(tensor=src_ap.tensor, offset=src_ap.offset,
                       ap=[[0, parts]] + list(src_ap.ap))

    with tile.TileContext(nc) as tc:
        with tc.tile_pool(name="P1", bufs=1) as P1, \
             tc.tile_pool(name="Pw", bufs=4) as Pw, \
             tc.tile_pool(name="Pst", bufs=2) as Pst, \
             tc.tile_pool(name="Psm", bufs=4) as Psm, \
             tc.tile_pool(name="Pgb", bufs=2) as Pgb, \
             tc.tile_pool(name="Phd", bufs=1) as Phd, \
             tc.tile_pool(name="Pps", bufs=2, space="PSUM") as Pps:

            # ---------- persistent tiles ----------
            x_res = P1.tile([128, NT, D], f32)
            xT8 = P1.tile([128, ND, T], f8)       # fp8(16*x^T) for DR linears
            xTb = P1.tile([128, ND, T], bf16)     # bf16 x^T for the FFN
            qT = P1.tile([128, ND, T], bf16)      # reused for q2T
            kT1 = P1.tile([128, ND, T], bf16)
            kTc = P1.tile([128, ND, T], bf16)
            vc = P1.tile([128, NT, H, 65], f16)
            v1 = P1.tile([128, NT, H, 65], f16)
            ident = P1.tile([128, 128], f32)
            bcut_sb = P1.tile([128, C - 1], f32)
            magic_t = P1.tile([128, 1], i32)
            nc.vector.memset(magic_t, MAGIC)
            ones1f = P1.tile([1, 128], f32)
            nc.vector.memset(ones1f, 1.0)
            ones1b = P1.tile([1, 128], bf16)
            nc.vector.tensor_copy(ones1b, ones1f)

            make_identity(nc, ident)
            nc.vector.memset(v1[:, :, :, 64:65], 1.0)

            # ---------- loop-invariant loads ----------
            nc.gpsimd.dma_start(kTc, ktc_d.rearrange("(jt p) t -> p jt t", p=128))
            nc.gpsimd.dma_start(vc, vac_d.rearrange("(tk p) j -> p tk j", p=128)
                                .rearrange("p tk (h e) -> p tk h e", h=H))
            nc.gpsimd.dma_start(bcut_sb, bcast(bcut_d))

            gb_idx = [0]

            def load_gb():
                pair = None
                if has_gb:
                    g_b = Pgb.tile([128, D], f32, tag="g_b", bufs=3)
                    b_b = Pgb.tile([128, D], f32, tag="b_b", bufs=3)
                    nc.sync.dma_start(g_b, bcast(gb_d[gb_idx[0] % (3 * L - 1), 0, :]))
                    nc.sync.dma_start(b_b, bcast(gb_d[gb_idx[0] % (3 * L - 1), 1, :]))
                    pair = (g_b, b_b)
                gb_idx[0] += 1
                return pair

            def layernorm_tile(t, gbp=None):
                """x_res[:, t, :] <- LN(x_res[:, t, :]) in place."""
                st6 = Psm.tile([128, 2, 6], f32, tag="st6")
                xin = x_res[:, t, :].rearrange("p (n f) -> p n f", f=512)
                for sg in range(2):
                    nc.vector.bn_stats(st6[:, sg, :], xin[:, sg, :])
                mv = Psm.tile([128, 2], f32, tag="mv")
                nc.vector.bn_aggr(mv, st6)
                # rstd via magic-seed Newton on DVE (2 iters, fp32-grade)
                v1t = Psm.tile([128, 1], f32, tag="v1t")
                nc.vector.tensor_scalar_add(v1t, mv[:, 1:2], EPS)
                sh = Psm.tile([128, 1], i32, tag="sh")
                nc.vector.tensor_scalar(sh, v1t.bitcast(i32), 1, None,
                                        AL.arith_shift_right)
                y = Psm.tile([128, 1], f32, tag="y")
                nc.vector.scalar_tensor_tensor(
                    out=y.bitcast(i32), in0=magic_t, scalar=0,
                    in1=sh, op0=AL.bypass, op1=AL.subtract)
                for _ in range(2):
                    a = Psm.tile([128, 1], f32, tag="nta")
                    nc.vector.tensor_tensor(out=a, in0=y, in1=y, op=AL.mult)
                    b = Psm.tile([128, 1], f32, tag="ntb")
                    nc.vector.tensor_tensor(out=b, in0=a, in1=v1t, op=AL.mult)
                    c2 = Psm.tile([128, 1], f32, tag="ntc")
                    nc.vector.tensor_scalar(c2, b, -0.5, 1.5, AL.mult, AL.add)
                    y2 = Psm.tile([128, 1], f32, tag="y", name="y2")
                    nc.vector.tensor_tensor(out=y2, in0=y, in1=c2, op=AL.mult)
                    y = y2
                nc.vector.tensor_scalar(
                    x_res[:, t, :], x_res[:, t, :],
                    mv[:, 0:1], y, AL.subtract, AL.mult)
                if has_gb:
                    nc.vector.scalar_tensor_tensor(
                        out=x_res[:, t, :], in0=x_res[:, t, :], scalar=0.0,
                        in1=gbp[0], op0=AL.bypass, op1=AL.mult)
                    nc.vector.tensor_tensor(
                        out=x_res[:, t, :], in0=x_res[:, t, :], in1=gbp[1],
                        op=AL.add)

            def transpose_tile(t, mode, ceng="dve"):
                """xT8 or xTb column t <- transpose(x_res[:, t, :]).
                mode: 'fp8' (xT8) or 'bf16' (xTb). ceng='act' routes the
                PSUM->SBUF copies through the Activation engine (use at
                phase boundaries where Act would otherwise idle)."""
                for dh in range(2):
                    ps = Pps.tile([128, 512], f32, tag="big", bufs=4,
                                  name="psT")
                    for di in range(4):
                        dt = dh * 4 + di
                        nc.tensor.transpose(
                            ps[:, di * 128:(di + 1) * 128],
                            x_res[:, t, dt * 128:(dt + 1) * 128], ident)
                    psv = ps.rearrange("p (di e) -> p di e", e=128)
                    xT = xT8 if mode == "fp8" else xTb
                    dst = xT[:, dh * 4:(dh + 1) * 4, t * 128:(t + 1) * 128]
                    if ceng == "act":
                        nc.scalar.copy(dst, psv)
                    else:
                        nc.vector.tensor_copy(dst, psv)

            def build_xT(mode):
                for t in range(NT):
                    transpose_tile(t, mode)

            def wdma(w, wsrc, l, ch):
                nc.sync.dma_start(w.rearrange("p dt j -> p (dt j)"), wsrc[l, ch])

            def lin8(wsrc, l, ch, dest, eng):
                """dest[:, 2ch+j2, :] (bf16) = DS * (x @ W.T).T chunk via
                fp8 DoubleRow matmuls. eng: 'act' or 'dve' for the copy."""
                w = Pw.tile([128, ND, 256], f8, tag="w8", name="w8")
                wdma(w, wsrc, l, ch)
                for j2 in range(2):
                    jt = ch * 2 + j2
                    ps = Pps.tile([128, 512], f32, tag="big", bufs=4,
                                  name="psL")
                    for th in range(2):
                        for dtp in range(4):
                            nc.tensor.matmul(
                                ps[:, th * 256:(th + 1) * 256],
                                w[:, 2 * dtp:2 * dtp + 2,
                                  j2 * 128:(j2 + 1) * 128],
                                xT8[:, 2 * dtp:2 * dtp + 2,
                                    th * 256:(th + 1) * 256],
                                start=(dtp == 0), stop=(dtp == 3),
                                perf_mode=DR)
                    if eng == "act":
                        nc.scalar.mul(dest[:, jt, :], ps, DS)
                    elif eng == "pool":
                        nc.gpsimd.scalar_tensor_tensor(
                            out=dest[:, jt, :], in0=ps, scalar=DS,
                            in1=ps, op0=AL.mult, op1=AL.bypass)
                    else:
                        nc.vector.tensor_scalar(dest[:, jt, :], ps, DS, None,
                                                AL.mult)

            def lin8v(wsrc, l, ch):
                """v1[:, t, 4ch:4ch+4, 0:64] (f16) = DS * x @ Wv.T chunk."""
                w = Pw.tile([128, ND, 256], f8, tag="w8", name="w8v")
                wdma(w, wsrc, l, ch)
                for t in range(NT):
                    ps = Pps.tile([128, 256], f32, tag="n256", bufs=2,
                                  name="psV")
                    for dtp in range(4):
                        nc.tensor.matmul(
                            ps, xT8[:, 2 * dtp:2 * dtp + 2,
                                    t * 128:(t + 1) * 128],
                            w[:, 2 * dtp:2 * dtp + 2, :],
                            start=(dtp == 0), stop=(dtp == 3), perf_mode=DR)
                    nc.vector.tensor_scalar(
                        v1[:, t, ch * 4:(ch + 1) * 4, 0:64],
                        ps.rearrange("p (h e) -> p h e", e=64),
                        DS, None, AL.mult)

            # causal per-tk score widths (>=256 keeps f32 paths efficient and
            # avoids sub-tile waste; extra cols are later masked or unread)
            C_TQ0 = [0, 128, 256, 256]

            def scores_exp(eg, hg, kT, causal):
                """eg[:, hh, tk, :] = exp(q k^T / 32); diag blocks masked via
                gpsimd affine_select after the exp."""
                for hh in range(4):
                    h = hg * 4 + hh
                    po = (h % 2) * 64
                    jt = h // 2
                    for tk in range(NT):
                        tq0 = C_TQ0[tk] if causal else 0
                        width = T - tq0
                        ps = Pps.tile([128, 512], f32, tag="big", bufs=4,
                                      name="psS")
                        nc.tensor.matmul(
                            ps[:, 0:width],
                            kT[po:po + 64, jt, tk * 128:(tk + 1) * 128],
                            qT[po:po + 64, jt, tq0:T],
                            start=True, stop=True)
                        nc.scalar.activation(
                            eg[:, hh, tk, tq0:T], ps[:, 0:width],
                            AF.Exp, scale=ISCALE)
                        if causal:
                            nc.gpsimd.affine_select(
                                out=eg[:, hh, tk, tk * 128:(tk + 1) * 128],
                                in_=eg[:, hh, tk, tk * 128:(tk + 1) * 128],
                                compare_op=AL.is_ge, fill=0.0,
                                base=0, pattern=[[1, 128]],
                                channel_multiplier=-1)

            def av_chunk(eg, hg, tq, vv, causal):
                """x_res[:, tq, hg*256:(hg+1)*256] += softmax-normalized AV."""
                ntk = (tq + 1) if causal else NT
                op = Pps.tile([128, 4, 65], f32, tag="ogrp", bufs=2,
                              name="psO")
                for hh in range(4):
                    h = hg * 4 + hh
                    for tk in range(ntk):
                        nc.tensor.matmul(
                            op[:, hh, :],
                            eg[:, hh, tk, tq * 128:(tq + 1) * 128],
                            vv[:, tk, h, :],
                            start=(tk == 0), stop=(tk == ntk - 1))
                rec = Psm.tile([128, 4], f32, tag="rec", bufs=4, name="rec")
                nc.vector.reciprocal(rec, op[:, :, 64])
                rec_b = bass.AP(tensor=rec.tensor, offset=rec.offset,
                                ap=[rec.ap[0], rec.ap[1], [0, 64]])
                onrm = Pst.tile([128, 4, 64], f32, tag="onrm", name="onrm")
                nc.vector.scalar_tensor_tensor(
                    out=onrm, in0=op[:, :, 0:64], scalar=0.0,
                    in1=rec_b, op0=AL.bypass, op1=AL.mult)
                nc.gpsimd.tensor_tensor(
                    out=x_res[:, tq, hg * 256:(hg + 1) * 256],
                    in0=onrm.rearrange("p h e -> p (h e)"),
                    in1=x_res[:, tq, hg * 256:(hg + 1) * 256],
                    op=AL.add)

            pd = probs_d.rearrange("(t p) (hf dd) c -> t hf p dd c",
                                   p=128, hf=4)

            def head_tile(t):
                """ordinal sigmoid head for t-tile; diffs split DVE/GPSIMD."""
                for hf in range(4):
                    xs = x_res[:, t, hf * 256:(hf + 1) * 256]
                    pr = Phd.tile([128, 256, C], f32, tag="probs",
                                  bufs=2, name="pr")
                    sprev = None
                    for c in range(C - 1):
                        eng = nc.vector if c % 2 == 0 else nc.gpsimd
                        scur = Pst.tile([128, 256], f32, tag="sig",
                                        bufs=4, name="sig")
                        nc.scalar.activation(
                            scur, xs, AF.Sigmoid, scale=-1.0,
                            bias=bcut_sb[:, c:c + 1])
                        if c == 0:
                            eng.tensor_copy(pr[:, :, 0], scur)
                        else:
                            eng.tensor_tensor(
                                out=pr[:, :, c], in0=scur, in1=sprev,
                                op=AL.subtract)
                        sprev = scur
                    nc.vector.tensor_scalar(pr[:, :, C - 1], sprev,
                                            -1.0, 1.0, AL.mult, AL.add)
                    nc.sync.dma_start(pd[t, hf], pr)

            def body():
                nc.sync.dma_start(x_res, x_d.rearrange("(t p) d -> p t d", p=128))
                build_xT("fp8")
                for l in range(L):
                    # ---- self-attention, linears interleaved per head-group ----
                    egs = []
                    for ch in range(4):
                        eg = Pst.tile([128, 4, NT, T], f16, tag="expg",
                                      bufs=4, name="expg")
                        egs.append(eg)
                        lin8(w8_d["q1"], l, ch, qT, "dve")
                        lin8(w8_d["k1"], l, ch, kT1, "dve")
                        lin8v(w8_d["v1"], l, ch)
                        scores_exp(eg, ch, kT1, causal=False)
                        if ch > 0:
                            for tqi in range(NT):
                                av_chunk(egs[ch - 1], ch - 1,
                                         (tqi + ch - 1) % NT, v1, False)
                    gb2 = load_gb()   # LN2 gamma/beta (None when !has_gb)
                    for tqi in range(NT):
                        tq = (tqi + 3) % NT
                        av_chunk(egs[3], 3, tq, v1, False)
                        layernorm_tile(tq, gb2)
                        transpose_tile(tq, "fp8")

                    # ---- cross-attention (tk-major exp) + fused LN3/FFN ----
                    for ch in range(4):
                        lin8(w8_d["q2"], l, ch, qT, "dve")
                    egs = [Pst.tile([128, 4, NT, T], f16, tag="expg",
                                    bufs=4, name="expgc") for _ in range(4)]
                    wfs = []
                    for ch in range(4):
                        w = Pw.tile([128, ND, 256], bf16, tag="wf", bufs=4,
                                    name="wf")
                        wdma(w, wf_d, l, ch)
                        wfs.append(w)
                    bfc_row = Pgb.tile([1, D], bf16, tag="bfc_row",
                                       name="bfc_row")
                    nc.sync.dma_start(
                        bfc_row,
                        bass.AP(tensor=bfc_d.tensor, offset=bfc_d[l].offset,
                                ap=[[0, 1], [1, D]]))
                    gb3 = load_gb()   # LN3 gamma/beta
                    gb1n = load_gb() if l < L - 1 else None  # LN1 of next layer
                    def cross_row(tk):
                        tq0 = C_TQ0[tk]
                        for hg in range(4):
                            for hh in range(4):
                                h = hg * 4 + hh
                                po = (h % 2) * 64
                                jt = h // 2
                                ps = Pps.tile([128, 512], f32, tag="big",
                                              bufs=4, name="psSc")
                                nc.tensor.matmul(
                                    ps[:, 0:T - tq0],
                                    kTc[po:po + 64, jt,
                                        tk * 128:(tk + 1) * 128],
                                    qT[po:po + 64, jt, tq0:T],
                                    start=True, stop=True)
                                e0 = tk * 128 - tq0   # first col the AV reads
                                nc.scalar.activation(
                                    egs[hg][:, hh, tk, tq0 + e0:T],
                                    ps[:, e0:T - tq0], AF.Exp, scale=ISCALE)
                                nc.gpsimd.affine_select(
                                    out=egs[hg][:, hh, tk,
                                                tk * 128:(tk + 1) * 128],
                                    in_=egs[hg][:, hh, tk,
                                                tk * 128:(tk + 1) * 128],
                                    compare_op=AL.is_ge, fill=0.0,
                                    base=0, pattern=[[1, 128]],
                                    channel_multiplier=-1)

                    cross_row(0)
                    for tk in range(NT):
                        if tk + 1 < NT:
                            cross_row(tk + 1)
                        # all tk'<=tk exps ready for every head: AV for tq=tk
                        for hg in range(4):
                            av_chunk(egs[hg], hg, tk, vc, True)
                        layernorm_tile(tk, gb3)
                        transpose_tile(tk, "bf16")
                        # ---- FFN for t-column tk ----
                        t = tk
                        for ch in range(4):
                            ps = Pps.tile([128, 256], f32, tag="n256",
                                          bufs=2, name="psF")
                            for dt in range(ND):
                                nc.tensor.matmul(
                                    ps, xTb[:, dt, t * 128:(t + 1) * 128],
                                    wfs[ch][:, dt, :],
                                    start=(dt == 0), stop=False)
                            nc.tensor.matmul(
                                ps, ones1b,
                                bfc_row[:, ch * 256:(ch + 1) * 256],
                                start=False, stop=True)
                            nc.vector.scalar_tensor_tensor(
                                out=x_res[:, t, ch * 256:(ch + 1) * 256],
                                in0=ps, scalar=0.0,
                                in1=x_res[:, t, ch * 256:(ch + 1) * 256],
                                op0=AL.max, op1=AL.add)
                        if l < L - 1:
                            layernorm_tile(t, gb1n)
                            transpose_tile(t, "fp8")
                        else:
                            head_tile(t)


            if reps > 1:
                with tc.For_i(0, reps, 1):
                    body()
            else:
                body()

    nc.compile()
    return nc


def _prep(inputs):
    import ml_dtypes
    x = np.asarray(inputs["x"])
    k = np.asarray(inputs["k"])
    v = np.asarray(inputs["v"])
    pos = np.asarray(inputs["pos"])
    B = x.shape[0]

    xp = (x + pos[None]).astype(np.float32)                       # [B,T,D]
    ktc = np.ascontiguousarray(
        k.transpose(0, 1, 3, 2).reshape(B, H * DH, T)).astype(ml_dtypes.bfloat16)
    va = np.ones((B, T, H, 65), np.float16)
    va[..., :64] = v.transpose(0, 2, 1, 3)                        # [B,tk,h,e]
    va = va.reshape(B, T, H * 65)

    def chunked(wkey, scale, dt):
        # W [L, j, d] -> W.T [L, d, j] -> chunked [L, ch, p, (dtile jj)]
        wT = np.asarray(inputs[wkey]).transpose(0, 2, 1).astype(np.float32)
        wc = (wT * scale).reshape(L, ND, 128, 4, 256).transpose(0, 3, 2, 1, 4)
        return np.ascontiguousarray(wc.reshape(L, 4, 128, ND * 256)).astype(dt)

    wt = {name: chunked(key, WS, ml_dtypes.float8_e4m3fn)
          for name, key in (("q1", "Wq1"), ("k1", "Wk1"),
                            ("v1", "Wv1"), ("q2", "Wq2"))}
    wt["fc"] = chunked("Wfc", 1.0, ml_dtypes.bfloat16)

    cut = np.asarray(inputs["cutoff"]).astype(np.float32)
    bcut = np.cumsum(
        np.concatenate([cut[:, :1], cut[:, 1:] ** 2], axis=1), axis=1)[0]  # [9]

    g1, b1 = np.asarray(inputs["g1"]), np.asarray(inputs["b1"])
    g2, b2 = np.asarray(inputs["g2"]), np.asarray(inputs["b2"])
    g3, b3 = np.asarray(inputs["g3"]), np.asarray(inputs["b3"])
    trivial = (np.all(g1 == 1) and np.all(g2 == 1) and np.all(g3 == 1)
               and np.all(b1 == 0) and np.all(b2 == 0) and np.all(b3 == 0))
    gb = None
    if not trivial:
        # order must match load_gb() call sites: per layer LN2, LN3, LN1(next)
        rows = []
        for l in range(L):
            rows.append((g2[l], b2[l]))
            rows.append((g3[l], b3[l]))
            if l < L - 1:
                rows.append((g1[l], b1[l]))
        gb = np.stack([np.stack(r) for r in rows]).astype(np.float32)

    bfc = np.asarray(inputs["bfc"]).astype(ml_dtypes.bfloat16)
    return xp, ktc, va, wt, bfc, bcut, gb, B


def kernel(**inputs):
    from concourse.bass_utils import run_bass_kernel_spmd

    xp, ktc, va, wt, bfc, bcut, gb, B = _prep(inputs)
    has_gb = gb is not None
    if ("nc", has_gb) not in _cache:
        _cache[("nc", has_gb)] = _build(has_gb)
    nc = _cache[("nc", has_gb)]

    in_maps = []
    for b in range(B):
        m = {
            "x": xp[b], "ktc": ktc[b], "vac": va[b],
            "wq1": wt["q1"], "wk1": wt["k1"], "wv1": wt["v1"],
            "wq2": wt["q2"], "wfc": wt["fc"],
            "bfc": bfc, "bcut": bcut,
        }
        if has_gb:
            m["gb"] = gb
        in_maps.append(m)

    res = run_bass_kernel_spmd(nc, in_maps, list(range(B)))
    out = np.stack([res.results[b]["probs"] for b in range(B)])
    return out.astype(np.float32)


# revision 14
# speedup vs baseline: 1.0438x; 1.0438x over previous
"""Trainium2 Bass kernel for nn_Decoder (dense transformer, B=8,T=512,D=1024,H=16,L=4,C=10).

Sharding: data-parallel over batch — core b computes batch element b end-to-end.

v2: fp8e4m3 DoubleRow matmuls for the q1/k1/v1/q2 projections (4x PE rate,
weights pre-scaled x512 on host, activations x16 via the transpose copies),
bf16 FFN, per-head-group interleave of linears with attention so the Exp
work on the Activation engine overlaps PE, causal mask via post-exp
affine_select on the (otherwise idle) GPSIMD engine, layernorms fused
per-t-tile into the attention/FFN tails.
"""
import sys
sys.path.insert(0, '/opt/trn_rl_repo')

import numpy as np

T, D, H, DH, L, C = 512, 1024, 16, 64, 4, 10
NT, ND = T // 128, D // 128          # 4 t-tiles, 8 d-tiles
EPS = 1e-5
ISCALE = 1.0 / 32.0                  # 1/sqrt(D)
MAGIC = 0x5F3759DF                   # rsqrt Newton seed
XS = 1.0                             # fp8 activation scale (unused: scale-free cast)
WS = 512.0                           # fp8 weight scale
DS = 1.0 / (XS * WS)                 # descale for fp8 linear outputs

_cache = {}


def _build(has_gb, reps=1):
    import contextlib
    import concourse.bass as bass
    from concourse import bacc, tile, mybir
    from concourse.masks import make_identity

    f32 = mybir.dt.float32
    f32r = mybir.dt.float32r
    bf16 = mybir.dt.bfloat16
    f16 = mybir.dt.float16
    f8 = mybir.dt.float8e4
    i32 = mybir.dt.int32
    AF = mybir.ActivationFunctionType
    AL = mybir.AluOpType
    DR = mybir.MatmulPerfMode.DoubleRow

    nc = bacc.Bacc("TRN2", target_bir_lowering=False, debug=False, num_devices=8)

    x_d = nc.dram_tensor("x", [T, D], f32, kind="ExternalInput").ap()
    ktc_d = nc.dram_tensor("ktc", [D, T], bf16, kind="ExternalInput").ap()
    vac_d = nc.dram_tensor("vac", [T, H * 65], f16, kind="ExternalInput").ap()
    # fp8 weights pre-arranged on host as [L, 4 chunks, 128 partitions, ND*256]
    w8_d = {
        m: nc.dram_tensor(f"w{m}", [L, 4, 128, ND * 256], f8,
                          kind="ExternalInput").ap()
        for m in ("q1", "k1", "v1", "q2")
    }
    wf_d = nc.dram_tensor("wfc", [L, 4, 128, ND * 256], bf16,
                          kind="ExternalInput").ap()
    bfc_d = nc.dram_tensor("bfc", [L, D], bf16, kind="ExternalInput").ap()
    bcut_d = nc.dram_tensor("bcut", [C - 1], f32, kind="ExternalInput").ap()
    if has_gb:
        gb_d = nc.dram_tensor("gb", [3 * L - 1, 2, D], f32, kind="ExternalInput").ap()
    probs_d = nc.dram_tensor("probs", [T, D, C], f32, kind="ExternalOutput").ap()

    def bcast(src_ap, parts=128):
        return bass.AP(tensor=src_ap.tensor, offset=src_ap.offset,
                       ap=[[0, parts]] + list(src_ap.ap))

    with tile.TileContext(nc) as tc:
        with tc.tile_pool(name="P1", bufs=1) as P1, \
             tc.tile_pool(name="Pw", bufs=4) as Pw, \
             tc.tile_pool(name="Pst", bufs=2) as Pst, \
             tc.tile_pool(name="Psm", bufs=4) as Psm, \
             tc.tile_pool(name="Pgb", bufs=2) as Pgb, \
             tc.tile_pool(name="Phd", bufs=1) as Phd, \
             tc.tile_pool(name="Pps", bufs=2, space="PSUM") as Pps:

            # ---------- persistent tiles ----------
            x_res = P1.tile([128, NT, D], f32)
            xT8 = P1.tile([128, ND, T], f8)       # fp8(16*x^T) for DR linears
            xTb = P1.tile([128, ND, T], bf16)     # bf16 x^T for the FFN
            qT = P1.tile([128, ND, T], bf16)      # reused for q2T
            kT1 = P1.tile([128, ND, T], bf16)
            kTc = P1.tile([128, ND, T], bf16)
            vc = P1.tile([128, NT, H, 65], f16)
            v1 = P1.tile([128, NT, H, 65], f16)
            ident = P1.tile([128, 128], f32)
            bcut_sb = P1.tile([128, C - 1], f32)
            magic_t = P1.tile([128, 1], i32)
            nc.vector.memset(magic_t, MAGIC)
            ones1f = P1.tile([1, 128], f32)
            nc.vector.memset(ones1f, 1.0)
            ones1b = P1.tile([1, 128], bf16)
            nc.vector.tensor_copy(ones1b, ones1f)

            make_identity(nc, ident)
            nc.vector.memset(v1[:, :, :, 64:65], 1.0)

            # ---------- loop-invariant loads ----------
            nc.gpsimd.dma_start(kTc, ktc_d.rearrange("(jt p) t -> p jt t", p=128))
            nc.gpsimd.dma_start(vc, vac_d.rearrange("(tk p) j -> p tk j", p=128)
                                .rearrange("p tk (h e) -> p tk h e", h=H))
            nc.gpsimd.dma_start(bcut_sb, bcast(bcut_d))

            gb_idx = [0]

            def load_gb():
                pair = None
                if has_gb:
                    g_b = Pgb.tile([128, D], f32, tag="g_b", bufs=3)
                    b_b = Pgb.tile([128, D], f32, tag="b_b", bufs=3)
                    nc.sync.dma_start(g_b, bcast(gb_d[gb_idx[0] % (3 * L - 1), 0, :]))
                    nc.sync.dma_start(b_b, bcast(gb_d[gb_idx[0] % (3 * L - 1), 1, :]))
                    pair = (g_b, b_b)
                gb_idx[0] += 1
                return pair

            def layernorm_tile(t, gbp=None):
                """x_res[:, t, :] <- LN(x_res[:, t, :]) in place."""
                st6 = Psm.tile([128, 2, 6], f32, tag="st6")
                xin = x_res[:, t, :].rearrange("p (n f) -> p n f", f=512)
                for sg in range(2):
                    nc.vector.bn_stats(st6[:, sg, :], xin[:, sg, :])
                mv = Psm.tile([128, 2], f32, tag="mv")
                nc.vector.bn_aggr(mv, st6)
                # rstd via magic-seed Newton on DVE (2 iters, fp32-grade)
                v1t = Psm.tile([128, 1], f32, tag="v1t")
                nc.vector.tensor_scalar_add(v1t, mv[:, 1:2], EPS)
                sh = Psm.tile([128, 1], i32, tag="sh")
                nc.vector.tensor_scalar(sh, v1t.bitcast(i32), 1, None,
                                        AL.arith_shift_right)
                y = Psm.tile([128, 1], f32, tag="y")
                nc.vector.scalar_tensor_tensor(
                    out=y.bitcast(i32), in0=magic_t, scalar=0,
                    in1=sh, op0=AL.bypass, op1=AL.subtract)
                for _ in range(2):
                    a = Psm.tile([128, 1], f32, tag="nta")
                    nc.vector.tensor_tensor(out=a, in0=y, in1=y, op=AL.mult)
                    b = Psm.tile([128, 1], f32, tag="ntb")
                    nc.vector.tensor_tensor(out=b, in0=a, in1=v1t, op=AL.mult)
                    c2 = Psm.tile([128, 1], f32, tag="ntc")
                    nc.vector.tensor_scalar(c2, b, -0.5, 1.5, AL.mult, AL.add)
                    y2 = Psm.tile([128, 1], f32, tag="y", name="y2")
                    nc.vector.tensor_tensor(out=y2, in0=y, in1=c2, op=AL.mult)
                    y = y2
                nc.vector.tensor_scalar(
                    x_res[:, t, :], x_res[:, t, :],
                    mv[:, 0:1], y, AL.subtract, AL.mult)
                if has_gb:
                    nc.vector.scalar_tensor_tensor(
                        out=x_res[:, t, :], in0=x_res[:, t, :], scalar=0.0,
                        in1=gbp[0], op0=AL.bypass, op1=AL.mult)
                    nc.vector.tensor_tensor(
                        out=x_res[:, t, :], in0=x_res[:, t, :], in1=gbp[1],
                        op=AL.add)

            def transpose_tile(t, mode, ceng="dve"):
                """xT8 or xTb column t <- transpose(x_res[:, t, :]).
                mode: 'fp8' (xT8) or 'bf16' (xTb). ceng='act' routes the
                PSUM->SBUF copies through the Activation engine (use at
                phase boundaries where Act would otherwise idle)."""
                for dh in range(2):
                    ps = Pps.tile([128, 512], f32, tag="big", bufs=4,
                                  name="psT")
                    for di in range(4):
                        dt = dh * 4 + di
                        nc.tensor.transpose(
                            ps[:, di * 128:(di + 1) * 128],
                            x_res[:, t, dt * 128:(dt + 1) * 128], ident)
                    psv = ps.rearrange("p (di e) -> p di e", e=128)
                    xT = xT8 if mode == "fp8" else xTb
                    dst = xT[:, dh * 4:(dh + 1) * 4, t * 128:(t + 1) * 128]
                    if ceng == "act":
                        nc.scalar.copy(dst, psv)
                    else:
                        nc.vector.tensor_copy(dst, psv)

            def build_xT(mode):
                for t in range(NT):
                    transpose_tile(t, mode)

            def wdma(w, wsrc, l, ch):
                nc.sync.dma_start(w.rearrange("p dt j -> p (dt j)"), wsrc[l, ch])

            def lin8(wsrc, l, ch, dest, eng):
                """dest[:, 2ch+j2, :] (bf16) = DS * (x @ W.T).T chunk via
                fp8 DoubleRow matmuls. eng: 'act' or 'dve' for the copy."""
                w = Pw.tile([128, ND, 256], f8, tag="w8", name="w8")
                wdma(w, wsrc, l, ch)
                for j2 in range(2):
                    jt = ch * 2 + j2
                    ps = Pps.tile([128, 512], f32, tag="big", bufs=4,
                                  name="psL")
                    for th in range(2):
                        for dtp in range(4):
                            nc.tensor.matmul(
                                ps[:, th * 256:(th + 1) * 256],
                                w[:, 2 * dtp:2 * dtp + 2,
                                  j2 * 128:(j2 + 1) * 128],
                                xT8[:, 2 * dtp:2 * dtp + 2,
                                    th * 256:(th + 1) * 256],
                                start=(dtp == 0), stop=(dtp == 3),
                                perf_mode=DR)
                    if eng == "act":
                        nc.scalar.mul(dest[:, jt, :], ps, DS)
                    elif eng == "pool":
                        nc.gpsimd.scalar_tensor_tensor(
                            out=dest[:, jt, :], in0=ps, scalar=DS,
                            in1=ps, op0=AL.mult, op1=AL.bypass)
                    else:
                        nc.vector.tensor_scalar(dest[:, jt, :], ps, DS, None,
                                                AL.mult)

            def lin8v(wsrc, l, ch):
                """v1[:, t, 4ch:4ch+4, 0:64] (f16) = DS * x @ Wv.T chunk."""
                w = Pw.tile([128, ND, 256], f8, tag="w8", name="w8v")
                wdma(w, wsrc, l, ch)
                for t in range(NT):
                    ps = Pps.tile([128, 256], f32, tag="n256", bufs=2,
                                  name="psV")
                    for dtp in range(4):
                        nc.tensor.matmul(
                            ps, xT8[:, 2 * dtp:2 * dtp + 2,
                                    t * 128:(t + 1) * 128],
                            w[:, 2 * dtp:2 * dtp + 2, :],
                            start=(dtp == 0), stop=(dtp == 3), perf_mode=DR)
                    nc.vector.tensor_scalar(
                        v1[:, t, ch * 4:(ch + 1) * 4, 0:64],
                        ps.rearrange("p (h e) -> p h e", e=64),
                        DS, None, AL.mult)

            # causal per-tk score widths (>=256 keeps f32 paths efficient and
            # avoids sub-tile waste; extra cols are later masked or unread)
            C_TQ0 = [0, 128, 256, 256]

            def scores_exp(eg, hg, kT, causal):
                """eg[:, hh, tk, :] = exp(q k^T / 32); diag blocks masked via
                gpsimd affine_select after the exp."""
                for hh in range(4):
                    h = hg * 4 + hh
                    po = (h % 2) * 64
                    jt = h // 2
                    for tk in range(NT):
                        tq0 = C_TQ0[tk] if causal else 0
                        width = T - tq0
                        ps = Pps.tile([128, 512], f32, tag="big", bufs=4,
                                      name="psS")
                        nc.tensor.matmul(
                            ps[:, 0:width],
                            kT[po:po + 64, jt, tk * 128:(tk + 1) * 128],
                            qT[po:po + 64, jt, tq0:T],
                            start=True, stop=True)
                        nc.scalar.activation(
                            eg[:, hh, tk, tq0:T], ps[:, 0:width],
                            AF.Exp, scale=ISCALE)
                        if causal:
                            nc.gpsimd.affine_select(
                                out=eg[:, hh, tk, tk * 128:(tk + 1) * 128],
                                in_=eg[:, hh, tk, tk * 128:(tk + 1) * 128],
                                compare_op=AL.is_ge, fill=0.0,
                                base=0, pattern=[[1, 128]],
                                channel_multiplier=-1)

            def av_chunk(eg, hg, tq, vv, causal):
                """x_res[:, tq, hg*256:(hg+1)*256] += softmax-normalized AV."""
                ntk = (tq + 1) if causal else NT
                op = Pps.tile([128, 4, 65], f32, tag="ogrp", bufs=2,
                              name="psO")
                for hh in range(4):
                    h = hg * 4 + hh
                    for tk in range(ntk):
                        nc.tensor.matmul(
                            op[:, hh, :],
                            eg[:, hh, tk, tq * 128:(tq + 1) * 128],
                            vv[:, tk, h, :],
                            start=(tk == 0), stop=(tk == ntk - 1))
                rec = Psm.tile([128, 4], f32, tag="rec", bufs=4, name="rec")
                nc.vector.reciprocal(rec, op[:, :, 64])
                rec_b = bass.AP(tensor=rec.tensor, offset=rec.offset,
                                ap=[rec.ap[0], rec.ap[1], [0, 64]])
                onrm = Pst.tile([128, 4, 64], f32, tag="onrm", name="onrm")
                nc.vector.scalar_tensor_tensor(
                    out=onrm, in0=op[:, :, 0:64], scalar=0.0,
                    in1=rec_b, op0=AL.bypass, op1=AL.mult)
                nc.gpsimd.tensor_tensor(
                    out=x_res[:, tq, hg * 256:(hg + 1) * 256],
                    in0=onrm.rearrange("p h e -> p (h e)"),
                    in1=x_res[:, tq, hg * 256:(hg + 1) * 256],
                    op=AL.add)

            pd = probs_d.rearrange("(t p) (hf dd) c -> t hf p dd c",
                                   p=128, hf=4)

            def head_tile(t):
                """ordinal sigmoid head for t-tile; diffs split DVE/GPSIMD."""
                for hf in range(4):
                    xs = x_res[:, t, hf * 256:(hf + 1) * 256]
                    pr = Phd.tile([128, 256, C], f32, tag="probs",
                                  bufs=2, name="pr")
                    sprev = None
                    for c in range(C - 1):
                        eng = nc.vector if c % 2 == 0 else nc.gpsimd
                        scur = Pst.tile([128, 256], f32, tag="sig",
                                        bufs=4, name="sig")
                        nc.scalar.activation(
                            scur, xs, AF.Sigmoid, scale=-1.0,
                            bias=bcut_sb[:, c:c + 1])
                        if c == 0:
                            eng.tensor_copy(pr[:, :, 0], scur)
                        else:
                            eng.tensor_tensor(
                                out=pr[:, :, c], in0=scur, in1=sprev,
                                op=AL.subtract)
                        sprev = scur
                    nc.vector.tensor_scalar(pr[:, :, C - 1], sprev,
                                            -1.0, 1.0, AL.mult, AL.add)
                    nc.sync.dma_start(pd[t, hf], pr)

            def body():
                nc.sync.dma_start(x_res, x_d.rearrange("(t p) d -> p t d", p=128))
                build_xT("fp8")
                for l in range(L):
                    # ---- self-attention, linears interleaved per head-group ----
                    egs = []
                    for ch in range(4):
                        eg = Pst.tile([128, 4, NT, T], f16, tag="expg",
                                      bufs=4, name="expg")
                        egs.append(eg)
                        lin8(w8_d["q1"], l, ch, qT, "dve")
                        lin8(w8_d["k1"], l, ch, kT1, "dve")
                        lin8v(w8_d["v1"], l, ch)
                        scores_exp(eg, ch, kT1, causal=False)
                        if ch > 0:
                            for tqi in range(NT):
                                av_chunk(egs[ch - 1], ch - 1,
                                         (tqi + ch - 1) % NT, v1, False)
                    gb2 = load_gb()   # LN2 gamma/beta (None when !has_gb)
                    for tqi in range(NT):
                        tq = (tqi + 3) % NT
                        av_chunk(egs[3], 3, tq, v1, False)
                        layernorm_tile(tq, gb2)
                        transpose_tile(tq, "fp8")

                    # ---- cross-attention (tk-major exp) + fused LN3/FFN ----
                    egs = [Pst.tile([128, 4, NT, T], f16, tag="expg",
                                    bufs=4, name="expgc") for _ in range(4)]
                    wfs = []
                    for ch in range(4):
                        w = Pw.tile([128, ND, 256], bf16, tag="wf", bufs=4,
                                    name="wf")
                        wdma(w, wf_d, l, ch)
                        wfs.append(w)
                    bfc_row = Pgb.tile([1, D], bf16, tag="bfc_row",
                                       name="bfc_row")
                    nc.sync.dma_start(
                        bfc_row,
                        bass.AP(tensor=bfc_d.tensor, offset=bfc_d[l].offset,
                                ap=[[0, 1], [1, D]]))
                    gb3 = load_gb()   # LN3 gamma/beta
                    gb1n = load_gb() if l < L - 1 else None  # LN1 of next layer
                    def cross_row(tk, hgs=(0, 1, 2, 3)):
                        tq0 = C_TQ0[tk]
                        for hg in hgs:
                            for hh in range(4):
                                h = hg * 4 + hh
                                po = (h % 2) * 64
                                jt = h // 2
                                ps = Pps.tile([128, 512], f32, tag="big",
                                              bufs=4, name="psSc")
                                nc.tensor.matmul(
                                    ps[:, 0:T - tq0],
                                    kTc[po:po + 64, jt,
                                        tk * 128:(tk + 1) * 128],
                                    qT[po:po + 64, jt, tq0:T],
                                    start=True, stop=True)
                                e0 = tk * 128 - tq0   # first col the AV reads
                                nc.scalar.activation(
                                    egs[hg][:, hh, tk, tq0 + e0:T],
                                    ps[:, e0:T - tq0], AF.Exp, scale=ISCALE)
                                nc.gpsimd.affine_select(
                                    out=egs[hg][:, hh, tk,
                                                tk * 128:(tk + 1) * 128],
                                    in_=egs[hg][:, hh, tk,
                                                tk * 128:(tk + 1) * 128],
                                    compare_op=AL.is_ge, fill=0.0,
                                    base=0, pattern=[[1, 128]],
                                    channel_multiplier=-1)

                    for ch in range(4):
                        lin8(w8_d["q2"], l, ch, qT, "dve")
                        cross_row(0, (ch,))   # row-0 scores as soon as this
                                              # head-group's q2 lands
                    for tk in range(NT):
                        if tk + 1 < NT:
                            cross_row(tk + 1)
                        # all tk'<=tk exps ready for every head: AV for tq=tk
                        for hg in range(4):
                            av_chunk(egs[hg], hg, tk, vc, True)
                        layernorm_tile(tk, gb3)
                        transpose_tile(tk, "bf16")
                        # ---- FFN for t-column tk ----
                        t = tk
                        for ch in range(4):
                            ps = Pps.tile([128, 256], f32, tag="n256",
                                          bufs=2, name="psF")
                            for dt in range(ND):
                                nc.tensor.matmul(
                                    ps, xTb[:, dt, t * 128:(t + 1) * 128],
                                    wfs[ch][:, dt, :],
                                    start=(dt == 0), stop=False)
                            nc.tensor.matmul(
                                ps, ones1b,
                                bfc_row[:, ch * 256:(ch + 1) * 256],
                                start=False, stop=True)
                            nc.vector.scalar_tensor_tensor(
                                out=x_res[:, t, ch * 256:(ch + 1) * 256],
                                in0=ps, scalar=0.0,
                                in1=x_res[:, t, ch * 256:(ch + 1) * 256],
                                op0=AL.max, op1=AL.add)
                        if l < L - 1:
                            layernorm_tile(t, gb1n)
                            transpose_tile(t, "fp8")
                        else:
                            head_tile(t)


            if reps > 1:
                with tc.For_i(0, reps, 1):
                    body()
            else:
                body()

    nc.compile()
    return nc


def _prep(inputs):
    import ml_dtypes
    x = np.asarray(inputs["x"])
    k = np.asarray(inputs["k"])
    v = np.asarray(inputs["v"])
    pos = np.asarray(inputs["pos"])
    B = x.shape[0]

    xp = (x + pos[None]).astype(np.float32)                       # [B,T,D]
    ktc = np.ascontiguousarray(
        k.transpose(0, 1, 3, 2).reshape(B, H * DH, T)).astype(ml_dtypes.bfloat16)
    va = np.ones((B, T, H, 65), np.float16)
    va[..., :64] = v.transpose(0, 2, 1, 3)                        # [B,tk,h,e]
    va = va.reshape(B, T, H * 65)

    def chunked(wkey, scale, dt):
        # W [L, j, d] -> W.T [L, d, j] -> chunked [L, ch, p, (dtile jj)]
        wT = np.asarray(inputs[wkey]).transpose(0, 2, 1).astype(np.float32)
        wc = (wT * scale).reshape(L, ND, 128, 4, 256).transpose(0, 3, 2, 1, 4)
        return np.ascontiguousarray(wc.reshape(L, 4, 128, ND * 256)).astype(dt)

    wt = {name: chunked(key, WS, ml_dtypes.float8_e4m3fn)
          for name, key in (("q1", "Wq1"), ("k1", "Wk1"),
                            ("v1", "Wv1"), ("q2", "Wq2"))}
    wt["fc"] = chunked("Wfc", 1.0, ml_dtypes.bfloat16)

    cut = np.asarray(inputs["cutoff"]).astype(np.float32)
    bcut = np.cumsum(
        np.concatenate([cut[:, :1], cut[:, 1:] ** 2], axis=1), axis=1)[0]  # [9]

    g1, b1 = np.asarray(inputs["g1"]), np.asarray(inputs["b1"])
    g2, b2 = np.asarray(inputs["g2"]), np.asarray(inputs["b2"])
    g3, b3 = np.asarray(inputs["g3"]), np.asarray(inputs["b3"])
    trivial = (np.all(g1 == 1) and np.all(g2 == 1) and np.all(g3 == 1)
               and np.all(b1 == 0) and np.all(b2 == 0) and np.all(b3 == 0))
    gb = None
    if not trivial:
        # order must match load_gb() call sites: per layer LN2, LN3, LN1(next)
        rows = []
        for l in range(L):
            rows.append((g2[l], b2[l]))
            rows.append((g3[l], b3[l]))
            if l < L - 1:
                rows.append((g1[l], b1[l]))
        gb = np.stack([np.stack(r) for r in rows]).astype(np.float32)

    bfc = np.asarray(inputs["bfc"]).astype(ml_dtypes.bfloat16)
    return xp, ktc, va, wt, bfc, bcut, gb, B


def kernel(**inputs):
    from concourse.bass_utils import run_bass_kernel_spmd

    xp, ktc, va, wt, bfc, bcut, gb, B = _prep(inputs)
    has_gb = gb is not None
    if ("nc", has_gb) not in _cache:
        _cache[("nc", has_gb)] = _build(has_gb)
    nc = _cache[("nc", has_gb)]

    in_maps = []
    for b in range(B):
        m = {
            "x": xp[b], "ktc": ktc[b], "vac": va[b],
            "wq1": wt["q1"], "wk1": wt["k1"], "wv1": wt["v1"],
            "wq2": wt["q2"], "wfc": wt["fc"],
            "bfc": bfc, "bcut": bcut,
        }
        if has_gb:
            m["gb"] = gb
        in_maps.append(m)

    res = run_bass_kernel_spmd(nc, in_maps, list(range(B)))
    out = np.stack([res.results[b]["probs"] for b in range(B)])
    return out.astype(np.float32)
